# revision 1
# baseline (speedup 1.0000x reference)
# DenseAtt kernel for Trainium2, 8 NeuronCores.
#   out[i, j] = adj[i, j] * sigmoid(x[i] @ W[:F] + x[j] @ W[F:] + b)
# Row-sharded: core c owns rows [c*1024, (c+1)*1024).
import numpy as np

import concourse.bass as bass
import concourse.tile as tile
from concourse import bacc, mybir
from concourse.bass_utils import run_bass_kernel_spmd
from concourse.masks import make_identity

N = 8192
F = 256
NCORES = 8
RPC = N // NCORES          # rows per core (1024)
RCHUNKS = RPC // 128       # row chunks of 128 per core (8)
XBLKS = N // 1024          # x loaded in blocks of 1024 rows (8)
CT = 4096                  # column tile of the main loop
NCT = N // CT              # column tiles per row chunk (2)

f32 = mybir.dt.float32

LAST_EXEC_NS = None
_CACHE = {}


def _build():
    nc = bacc.Bacc(
        "TRN2", target_bir_lowering=False, debug=False,
        enable_asserts=True, num_devices=NCORES,
    )
    adj_s = nc.dram_tensor("adj_s", (RPC, N), f32, kind="ExternalInput").ap()
    x_all = nc.dram_tensor("x_all", (N, F), f32, kind="ExternalInput").ap()
    x_own = nc.dram_tensor("x_own", (RPC, F), f32, kind="ExternalInput").ap()
    w_in = nc.dram_tensor("w_in", (1, 2 * F), f32, kind="ExternalInput").ap()
    b_in = nc.dram_tensor("b_in", (1, 1), f32, kind="ExternalInput").ap()
    out_s = nc.dram_tensor("out_s", (RPC, N), f32, kind="ExternalOutput").ap()

    AF = mybir.ActivationFunctionType
    OP = mybir.AluOpType

    with tile.TileContext(nc) as tc:
        with (
            tc.tile_pool(name="const", bufs=1) as cpool,
            tc.tile_pool(name="xp", bufs=3) as xpool,
            tc.tile_pool(name="scr", bufs=2) as scrpool,
            tc.tile_pool(name="rbp", bufs=1) as rbpool,
            tc.tile_pool(name="adj", bufs=4) as adjpool,
            tc.tile_pool(name="att", bufs=3) as attpool,
            tc.tile_pool(name="mmps", bufs=2, space="PSUM") as pspool,
            tc.tile_pool(name="tpps", bufs=1, space="PSUM") as tppool,
            tc.tile_pool(name="dram", bufs=1, space="DRAM") as dpool,
        ):
            # ---- constants ----
            w_sb = cpool.tile([1, 2 * F], f32)
            nc.sync.dma_start(out=w_sb[:], in_=w_in)
            b_sb = cpool.tile([1, 1], f32)
            nc.sync.dma_start(out=b_sb[:], in_=b_in)
            ones = cpool.tile([1, 128], f32)
            nc.vector.memset(ones[:], 1.0)
            ident = cpool.tile([128, 128], f32)
            make_identity(nc, ident[:])

            # ---- broadcast W and b across all 128 partitions (K=1 matmul) ----
            wb_ps = pspool.tile([128, 512], f32, tag="mm")
            nc.tensor.matmul(wb_ps[:], ones[:], w_sb[:], start=True, stop=True)
            wb = cpool.tile([128, 2 * F], f32)
            nc.scalar.copy(wb[:], wb_ps[:])
            bb_ps = pspool.tile([128, 512], f32, tag="mm")
            nc.tensor.matmul(bb_ps[:, 0:1], ones[:], b_sb[:], start=True, stop=True)
            bb = cpool.tile([128, 1], f32)
            nc.scalar.copy(bb[:], bb_ps[:, 0:1])

            # ---- per-row dot products: R[p, t] = x[t*128+p] @ W[F:] ----
            R = cpool.tile([128, N // 128], f32)     # right, all rows
            L = cpool.tile([128, RCHUNKS], f32)      # left, own rows
            for blk in range(XBLKS):
                xt = xpool.tile([128, 8, F], f32, tag="xt")
                nc.sync.dma_start(
                    out=xt[:],
                    in_=x_all[blk * 1024:(blk + 1) * 1024].rearrange(
                        "(s p) f -> p s f", p=128),
                )
                for s in range(8):
                    t = blk * 8 + s
                    prod = scrpool.tile([128, F], f32, tag="prod")
                    nc.vector.scalar_tensor_tensor(
                        out=prod[:], in0=xt[:, s, :], scalar=1.0,
                        in1=wb[:, F:2 * F], op0=OP.mult, op1=OP.mult,
                        accum_out=R[:, t:t + 1],
                    )
            xo = xpool.tile([128, 8, F], f32, tag="xt")
            nc.sync.dma_start(
                out=xo[:], in_=x_own.rearrange("(s p) f -> p s f", p=128))
            for s in range(RCHUNKS):
                prod = scrpool.tile([128, F], f32, tag="prod")
                nc.vector.scalar_tensor_tensor(
                    out=prod[:], in0=xo[:, s, :], scalar=1.0,
                    in1=wb[:, 0:F], op0=OP.mult, op1=OP.mult,
                    accum_out=L[:, s:s + 1],
                )

            # ---- right row vector: transpose R, add b, roundtrip via DRAM ----
            rt_ps = tppool.tile([64, 128], f32, tag="tp")
            nc.tensor.transpose(rt_ps[:], R[:], ident[:])
            rt_sb = cpool.tile([64, 128], f32)
            nc.scalar.activation(rt_sb[:], rt_ps[:], AF.Identity, bias=bb[0:64])
            dsc = dpool.tile([1, N], f32)
            nc.gpsimd.dma_start(
                out=dsc[:].rearrange("o (t p) -> (o t) p", p=128), in_=rt_sb[:])
            rrow = cpool.tile([1, N], f32)
            nc.gpsimd.dma_start(out=rrow[:], in_=dsc[:])

            # ---- broadcast right+b across partitions: rb[i, j] = right[j]+b ----
            rb = rbpool.tile([128, N], f32)
            for i in range(N // 512):
                rb_ps = pspool.tile([128, 512], f32, tag="mm")
                nc.tensor.matmul(
                    rb_ps[:], ones[:], rrow[:, i * 512:(i + 1) * 512],
                    start=True, stop=True)
                nc.scalar.copy(rb[:, i * 512:(i + 1) * 512], rb_ps[:])

            # ---- main loop: att = sigmoid(rb + left); out = adj * att ----
            for rc in range(RCHUNKS):
                for ct in range(NCT):
                    js = ct * CT
                    adj_t = adjpool.tile([128, CT], f32, tag="adj")
                    nc.sync.dma_start(
                        out=adj_t[:],
                        in_=adj_s[rc * 128:(rc + 1) * 128, js:js + CT])
                    att_t = attpool.tile([128, CT], f32, tag="att")
                    nc.scalar.activation(
                        att_t[:], rb[:, js:js + CT], AF.Sigmoid,
                        bias=L[:, rc:rc + 1])
                    nc.vector.tensor_mul(out=adj_t[:], in0=att_t[:], in1=adj_t[:])
                    nc.gpsimd.dma_start(
                        out=out_s[rc * 128:(rc + 1) * 128, js:js + CT],
                        in_=adj_t[:])

    nc.compile()
    return nc


def kernel(x, adj, W, b):
    global LAST_EXEC_NS
    if "nc" not in _CACHE:
        _CACHE["nc"] = _build()
    nc = _CACHE["nc"]

    x = np.ascontiguousarray(np.asarray(x, dtype=np.float32))
    adj = np.ascontiguousarray(np.asarray(adj, dtype=np.float32))
    w_in = np.ascontiguousarray(np.asarray(W, dtype=np.float32).reshape(1, 2 * F))
    b_in = np.ascontiguousarray(np.asarray(b, dtype=np.float32).reshape(1, 1))

    in_maps = []
    for c in range(NCORES):
        in_maps.append({
            "adj_s": np.ascontiguousarray(adj[c * RPC:(c + 1) * RPC]),
            "x_all": x,
            "x_own": np.ascontiguousarray(x[c * RPC:(c + 1) * RPC]),
            "w_in": w_in,
            "b_in": b_in,
        })
    res = run_bass_kernel_spmd(nc, in_maps, core_ids=list(range(NCORES)))
    LAST_EXEC_NS = res.exec_time_ns
    return np.concatenate([res.results[c]["out_s"] for c in range(NCORES)], axis=0)
